# revision 16
# baseline (speedup 1.0000x reference)
"""Trainium2 Bass kernel: per-point 3x3 Gaussian covariance from quaternion + log_scale.

cov = R diag(exp(log_scale)) R^T  with R built from the normalized quaternion.

Layout (per core): points sharded [128 partitions, R rows]; tiles of F points
per partition; all DMAs per-partition contiguous.  Normalization folded via
inv2 = 2/|q|^2 (computed fp32 as exp(-ln(n2/2))); the multiply-heavy chain
(products -> R -> M -> Gram) runs in bf16 with contiguous step-1 operands so
VectorE hits its 2x perf mode; ScalarE does the strided deinterleave/cast,
squares, exp/ln, and output interleave.
"""

import os
import numpy as np

import concourse.bass as bass
import concourse.bacc as bacc
import concourse.mybir as mybir
from concourse.tile import TileContext
from concourse.bass_utils import run_bass_kernel_spmd

AF = mybir.ActivationFunctionType
FP32 = mybir.dt.float32
BF16 = mybir.dt.bfloat16

N_CORES = 8
N_FULL = 4_000_000
P = 128
R = 3908                      # rows per partition per core; 128*3908*8 = 4_001_792 >= N
NPC = P * R                   # points per core (padded)
F = int(os.environ.get("KERNEL_F", "448"))  # points per partition per tile

SQRT_HALF = 0.7071067811865476

_built = {}


def _build():
    key = F
    if key in _built:
        return _built[key]

    nc = bacc.Bacc("TRN2", target_bir_lowering=False, debug=False, num_devices=N_CORES)
    q = nc.dram_tensor("q", [NPC, 4], FP32, kind="ExternalInput")
    ls = nc.dram_tensor("ls", [NPC, 3], FP32, kind="ExternalInput")
    cov = nc.dram_tensor("cov", [NPC, 3, 3], FP32, kind="ExternalOutput")

    qv = q.ap().rearrange("(p r) c -> p (r c)", p=P)       # [128, 4R]
    lsv = ls.ap().rearrange("(p r) c -> p (r c)", p=P)     # [128, 3R]
    ov = cov.ap().rearrange("(p r) i k -> p (r i k)", p=P)  # [128, 9R]

    with TileContext(nc) as tc:
        with (
            tc.tile_pool(name="io", bufs=2) as io,
            tc.tile_pool(name="otp", bufs=2) as ot_pool,
            tc.tile_pool(name="big", bufs=2) as big,
            tc.tile_pool(name="wk", bufs=2) as wk,
        ):
            t0 = 0
            while t0 < R:
                f = min(F, R - t0)
                _tile_body(nc, io, ot_pool, big, wk, qv, lsv, ov, t0, f)
                t0 += f

    nc.compile()
    _built[key] = nc
    return nc


def _tile_body(nc, io, ot_pool, big, wk, qv, lsv, ov, t0, f):
    cnt = [0]

    def w(dt=BF16, tag=None):
        cnt[0] += 1
        tag = tag or f"w{cnt[0]}"
        return wk.tile([P, f], dt, tag=tag, name=f"{tag}_t{t0}_{cnt[0]}")

    qt = io.tile([P, 4 * f], FP32, tag="qt", name=f"qt{t0}")
    lst = io.tile([P, 3 * f], FP32, tag="lst", name=f"lst{t0}")
    nc.sync.dma_start(out=qt, in_=qv[:, 4 * t0:4 * (t0 + f)])
    nc.sync.dma_start(out=lst, in_=lsv[:, 3 * t0:3 * (t0 + f)])

    qc = qt.rearrange("p (f c) -> p f c", c=4)
    lsc = lst.rearrange("p (f c) -> p f c", c=3)

    # ---- fp32 path: n2/2 and inv2 = 2/|q|^2 = exp(-ln(n2/2)) -------------
    sq4 = big.tile([P, 4 * f], FP32, tag="sq4", name=f"sq4_{t0}")
    nc.scalar.activation(sq4, qt, AF.Square, scale=SQRT_HALF)  # x^2/2
    sqc = sq4.rearrange("p (f c) -> p f c", c=4)
    u = w(FP32, tag="fu"); v = w(FP32, tag="fv"); n2h = w(FP32, tag="fn2h")
    lnv = w(FP32, tag="fu"); inv2 = w(FP32, tag="fv")
    nc.vector.tensor_add(u, sqc[:, :, 0], sqc[:, :, 1])
    nc.vector.tensor_add(v, sqc[:, :, 2], sqc[:, :, 3])
    nc.vector.tensor_add(n2h, u, v)
    nc.scalar.activation(lnv, n2h, AF.Ln)
    nc.scalar.activation(inv2, lnv, AF.Exp, scale=-1.0)

    # ---- deinterleave + cast to bf16 (ScalarE, strided reads) ------------
    a_ = w(); b_ = w(); c_ = w(); d_ = w(); ivb = w()
    nc.scalar.copy(out=a_, in_=qc[:, :, 0])
    nc.scalar.copy(out=b_, in_=qc[:, :, 1])
    nc.scalar.copy(out=c_, in_=qc[:, :, 2])
    nc.scalar.copy(out=d_, in_=qc[:, :, 3])
    nc.scalar.copy(out=ivb, in_=inv2)

    # ---- bf16 chain: A..D, products (VectorE 2x mode) --------------------
    A = w(); B = w(); C = w(); D = w()
    nc.vector.tensor_mul(A, ivb, a_)
    nc.vector.tensor_mul(B, ivb, b_)
    nc.vector.tensor_mul(C, ivb, c_)
    nc.vector.tensor_mul(D, ivb, d_)

    Ab = w(); Ac = w(); Ad = w()
    Bb = w(); Bc = w(); Bd = w()
    Cc = w(); Cd = w(); Dd = w()
    nc.vector.tensor_mul(Ab, A, b_)
    nc.vector.tensor_mul(Ac, A, c_)
    nc.vector.tensor_mul(Ad, A, d_)
    nc.vector.tensor_mul(Bb, B, b_)
    nc.vector.tensor_mul(Bc, B, c_)
    nc.vector.tensor_mul(Bd, B, d_)
    nc.vector.tensor_mul(Cc, C, c_)
    nc.vector.tensor_mul(Cd, C, d_)
    nc.vector.tensor_mul(Dd, D, d_)

    # ---- rotation matrix entries (bf16) ----------------------------------
    t_0 = w(); t_1 = w(); t_2 = w()
    nc.vector.tensor_add(t_0, Cc, Dd)
    nc.vector.tensor_add(t_1, Bb, Dd)
    nc.vector.tensor_add(t_2, Bb, Cc)
    r00 = w(FP32, tag="fr00"); r11 = w(FP32, tag="fr11"); r22 = w(FP32, tag="fr22")
    nc.scalar.activation(r00, t_0, AF.Identity, bias=1.0, scale=-1.0)
    nc.scalar.activation(r11, t_1, AF.Identity, bias=1.0, scale=-1.0)
    nc.scalar.activation(r22, t_2, AF.Identity, bias=1.0, scale=-1.0)
    r01 = w(); r10 = w(); r02 = w(); r20 = w(); r12 = w(); r21 = w()
    nc.vector.tensor_sub(r01, Bc, Ad)
    nc.vector.tensor_add(r10, Bc, Ad)
    nc.vector.tensor_add(r02, Bd, Ac)
    nc.vector.tensor_sub(r20, Bd, Ac)
    nc.vector.tensor_sub(r12, Cd, Ab)
    nc.vector.tensor_add(r21, Cd, Ab)

    # ---- sqrt(scale) per column (ScalarE, bf16 contiguous out) -----------
    sh = [w(FP32, tag="fsh0"), w(FP32, tag="fsh1"), w(FP32, tag="fsh2")]
    for j in range(3):
        nc.scalar.activation(sh[j], lsc[:, :, j], AF.Exp, scale=0.5)

    Rm = [[r00, r01, r02], [r10, r11, r12], [r20, r21, r22]]
    M = [[None] * 3 for _ in range(3)]
    for i in range(3):
        for j in range(3):
            M[i][j] = w(FP32 if i == j else BF16, tag=f"pm{i}{j}")
            nc.vector.tensor_mul(M[i][j], Rm[i][j], sh[j])

    # ---- cov = M M^T; diag entries write straight into the out tile ------
    ot = ot_pool.tile([P, 9 * f], FP32, tag="ot", name=f"ot_{t0}")
    otv = ot.rearrange("p (f e) -> p f e", e=9)
    offd = {}
    for (i, k) in [(0, 0), (0, 1), (0, 2), (1, 1), (1, 2), (2, 2)]:
        fd = i == k
        g = w(FP32 if fd else BF16, tag="ggf" if fd else "gg")
        g2 = w(FP32 if fd else BF16, tag="gg2f" if fd else "gg2")
        h = w(tag="gh"); h2 = w(tag="gh2")
        nc.vector.tensor_mul(g, M[i][0], M[k][0])
        nc.vector.tensor_mul(h, M[i][1], M[k][1])
        nc.vector.tensor_add(g2, g, h)
        nc.vector.tensor_mul(h2, M[i][2], M[k][2])
        if i == k:
            nc.vector.tensor_add(otv[:, :, 3 * i + k], g2, h2)  # fp32 strided out
        else:
            cik = w(tag=f"cov{i}{k}")
            nc.vector.tensor_add(cik, g2, h2)
            offd[(i, k)] = cik

    # off-diagonals + symmetric duplicates via ScalarE copies (cast to fp32)
    for (i, k), cik in offd.items():
        nc.scalar.copy(out=otv[:, :, 3 * i + k], in_=cik)
        nc.scalar.copy(out=otv[:, :, 3 * k + i], in_=cik)

    nc.sync.dma_start(out=ov[:, 9 * t0:9 * (t0 + f)], in_=ot)


def _pad_and_shard(quaternion, log_scale):
    n = quaternion.shape[0]
    pad = N_CORES * NPC - n
    if pad:
        qpad = np.tile(np.array([1, 0, 0, 0], np.float32), (pad, 1))
        lpad = np.zeros((pad, 3), np.float32)
        quaternion = np.concatenate([quaternion, qpad], axis=0)
        log_scale = np.concatenate([log_scale, lpad], axis=0)
    in_maps = []
    for i in range(N_CORES):
        sl = slice(i * NPC, (i + 1) * NPC)
        in_maps.append({
            "q": np.ascontiguousarray(quaternion[sl]),
            "ls": np.ascontiguousarray(log_scale[sl]),
        })
    return in_maps


def kernel_with_stats(quaternion, log_scale, trace=False):
    quaternion = np.asarray(quaternion, dtype=np.float32)
    log_scale = np.asarray(log_scale, dtype=np.float32)
    n = quaternion.shape[0]
    nc = _build()
    in_maps = _pad_and_shard(quaternion, log_scale)
    res = run_bass_kernel_spmd(nc, in_maps, core_ids=list(range(N_CORES)), trace=trace)
    out = np.concatenate([r["cov"] for r in res.results], axis=0)[:n]
    return out, res


def kernel(quaternion, log_scale):
    out, _ = kernel_with_stats(quaternion, log_scale, trace=False)
    return out


# revision 17
# speedup vs baseline: 1.0066x; 1.0066x over previous
"""Trainium2 Bass kernel: per-point 3x3 Gaussian covariance from quaternion + log_scale.

cov = R diag(exp(log_scale)) R^T  with R built from the normalized quaternion.

Layout (per core): points sharded [128 partitions, R rows]; tiles of F points
per partition; all DMAs per-partition contiguous.  Normalization folded via
inv2 = 2/|q|^2 (computed fp32 as exp(-ln(n2/2))); the multiply-heavy chain
(products -> R -> M -> Gram) runs in bf16 with contiguous step-1 operands so
VectorE hits its 2x perf mode; ScalarE does the strided deinterleave/cast,
squares, exp/ln, and output interleave.
"""

import os
import numpy as np

import concourse.bass as bass
import concourse.bacc as bacc
import concourse.mybir as mybir
from concourse.tile import TileContext
from concourse.bass_utils import run_bass_kernel_spmd

AF = mybir.ActivationFunctionType
FP32 = mybir.dt.float32
BF16 = mybir.dt.bfloat16

N_CORES = 8
N_FULL = 4_000_000
P = 128
R = 3908                      # rows per partition per core; 128*3908*8 = 4_001_792 >= N
NPC = P * R                   # points per core (padded)
F = int(os.environ.get("KERNEL_F", "448"))  # points per partition per tile

SQRT_HALF = 0.7071067811865476

_built = {}


def _build():
    key = F
    if key in _built:
        return _built[key]

    nc = bacc.Bacc("TRN2", target_bir_lowering=False, debug=False, num_devices=N_CORES)
    q = nc.dram_tensor("q", [NPC, 4], FP32, kind="ExternalInput")
    ls = nc.dram_tensor("ls", [NPC, 3], FP32, kind="ExternalInput")
    cov = nc.dram_tensor("cov", [NPC, 3, 3], FP32, kind="ExternalOutput")

    qv = q.ap().rearrange("(p r) c -> p (r c)", p=P)       # [128, 4R]
    lsv = ls.ap().rearrange("(p r) c -> p (r c)", p=P)     # [128, 3R]
    ov = cov.ap().rearrange("(p r) i k -> p (r i k)", p=P)  # [128, 9R]

    with TileContext(nc) as tc:
        with (
            tc.tile_pool(name="io", bufs=2) as io,
            tc.tile_pool(name="otp", bufs=2) as ot_pool,
            tc.tile_pool(name="big", bufs=2) as big,
            tc.tile_pool(name="wk", bufs=2) as wk,
        ):
            t0 = 0
            while t0 < R:
                f = min(F, R - t0)
                _tile_body(nc, io, ot_pool, big, wk, qv, lsv, ov, t0, f)
                t0 += f

    nc.compile()
    _built[key] = nc
    return nc


def _tile_body(nc, io, ot_pool, big, wk, qv, lsv, ov, t0, f):
    cnt = [0]

    def w(dt=BF16, tag=None):
        cnt[0] += 1
        tag = tag or f"w{cnt[0]}"
        return wk.tile([P, f], dt, tag=tag, name=f"{tag}_t{t0}_{cnt[0]}")

    qt = io.tile([P, 4 * f], FP32, tag="qt", name=f"qt{t0}")
    lst = io.tile([P, 3 * f], FP32, tag="lst", name=f"lst{t0}")
    nc.sync.dma_start(out=qt, in_=qv[:, 4 * t0:4 * (t0 + f)])
    nc.sync.dma_start(out=lst, in_=lsv[:, 3 * t0:3 * (t0 + f)])

    qc = qt.rearrange("p (f c) -> p f c", c=4)
    lsc = lst.rearrange("p (f c) -> p f c", c=3)

    # ---- fp32 path: n2/2 and inv2 = 2/|q|^2 = exp(-ln(n2/2)) -------------
    sq4 = big.tile([P, 4 * f], FP32, tag="sq4", name=f"sq4_{t0}")
    nc.scalar.activation(sq4, qt, AF.Square, scale=SQRT_HALF)  # x^2/2
    sqc = sq4.rearrange("p (f c) -> p f c", c=4)
    u = w(FP32, tag="fu"); v = w(FP32, tag="fv"); n2h = w(FP32, tag="fn2h")
    lnv = w(FP32, tag="fu"); inv2 = w(FP32, tag="fv")
    nc.vector.tensor_add(u, sqc[:, :, 0], sqc[:, :, 1])
    nc.vector.tensor_add(v, sqc[:, :, 2], sqc[:, :, 3])
    nc.vector.tensor_add(n2h, u, v)
    nc.scalar.activation(lnv, n2h, AF.Ln)
    nc.scalar.activation(inv2, lnv, AF.Exp, scale=-1.0)

    # ---- deinterleave + cast to bf16 (ScalarE, strided reads) ------------
    a_ = w(); b_ = w(); c_ = w(); d_ = w(); ivb = w()
    nc.scalar.copy(out=a_, in_=qc[:, :, 0])
    nc.scalar.copy(out=b_, in_=qc[:, :, 1])
    nc.scalar.copy(out=c_, in_=qc[:, :, 2])
    nc.scalar.copy(out=d_, in_=qc[:, :, 3])
    nc.scalar.copy(out=ivb, in_=inv2)

    # ---- bf16 chain: A..D, products (VectorE 2x mode) --------------------
    A = w(); B = w(); C = w(); D = w()
    nc.vector.tensor_mul(A, ivb, a_)
    nc.vector.tensor_mul(B, ivb, b_)
    nc.vector.tensor_mul(C, ivb, c_)
    nc.vector.tensor_mul(D, ivb, d_)

    Ab = w(); Ac = w(); Ad = w()
    Bb = w(); Bc = w(); Bd = w()
    Cc = w(); Cd = w(); Dd = w()
    nc.vector.tensor_mul(Ab, A, b_)
    nc.vector.tensor_mul(Ac, A, c_)
    nc.vector.tensor_mul(Ad, A, d_)
    nc.vector.tensor_mul(Bb, B, b_)
    nc.vector.tensor_mul(Bc, B, c_)
    nc.vector.tensor_mul(Bd, B, d_)
    nc.vector.tensor_mul(Cc, C, c_)
    nc.vector.tensor_mul(Cd, C, d_)
    nc.vector.tensor_mul(Dd, D, d_)

    # ---- rotation matrix entries (bf16) ----------------------------------
    t_0 = w(); t_1 = w(); t_2 = w()
    nc.vector.tensor_add(t_0, Cc, Dd)
    nc.vector.tensor_add(t_1, Bb, Dd)
    nc.vector.tensor_add(t_2, Bb, Cc)
    r00 = w(); r11 = w(); r22 = w()
    nc.scalar.activation(r00, t_0, AF.Identity, bias=1.0, scale=-1.0)
    nc.scalar.activation(r11, t_1, AF.Identity, bias=1.0, scale=-1.0)
    nc.scalar.activation(r22, t_2, AF.Identity, bias=1.0, scale=-1.0)
    r01 = w(); r10 = w(); r02 = w(); r20 = w(); r12 = w(); r21 = w()
    nc.vector.tensor_sub(r01, Bc, Ad)
    nc.vector.tensor_add(r10, Bc, Ad)
    nc.vector.tensor_add(r02, Bd, Ac)
    nc.vector.tensor_sub(r20, Bd, Ac)
    nc.vector.tensor_sub(r12, Cd, Ab)
    nc.vector.tensor_add(r21, Cd, Ab)

    # ---- sqrt(scale) per column (ScalarE, bf16 contiguous out) -----------
    sh = [w(tag="sh0"), w(tag="sh1"), w(tag="sh2")]
    for j in range(3):
        nc.scalar.activation(sh[j], lsc[:, :, j], AF.Exp, scale=0.5)

    Rm = [[r00, r01, r02], [r10, r11, r12], [r20, r21, r22]]
    M = [[None] * 3 for _ in range(3)]
    for i in range(3):
        for j in range(3):
            M[i][j] = w(tag=f"pm{i}{j}")
            nc.vector.tensor_mul(M[i][j], Rm[i][j], sh[j])

    # ---- full scale s_j = exp(ls_j), fp32 (diag leading term) ------------
    sf = [w(FP32, tag="sf0"), w(FP32, tag="sf1"), w(FP32, tag="sf2")]
    for j in range(3):
        nc.scalar.activation(sf[j], lsc[:, :, j], AF.Exp)

    ot = ot_pool.tile([P, 9 * f], FP32, tag="ot", name=f"ot_{t0}")
    otv = ot.rearrange("p (f e) -> p f e", e=9)

    # ---- diag: cov_ii = s_i + s_i*(t_i^2 - 2 t_i) + sum_{j!=i} M_ij^2 ----
    ts = [t_0, t_1, t_2]
    for i in range(3):
        sib = w(tag="sib")                       # s_i in bf16 = sh_i^2
        nc.vector.tensor_mul(sib, sh[i], sh[i])
        um = w(tag="um"); e_ = w(tag="e_"); c1 = w(tag="c1")
        nc.vector.tensor_scalar_add(um, ts[i], -2.0)     # t_i - 2
        nc.vector.tensor_mul(e_, ts[i], um)              # t_i^2 - 2 t_i
        nc.vector.tensor_mul(c1, sib, e_)
        j1, j2 = [j for j in range(3) if j != i]
        h = w(tag="gh"); h2 = w(tag="gh2"); hs = w(tag="ghs"); tot = w(tag="gtot")
        nc.vector.tensor_mul(h, M[i][j1], M[i][j1])
        nc.vector.tensor_mul(h2, M[i][j2], M[i][j2])
        nc.vector.tensor_add(hs, h, h2)
        nc.vector.tensor_add(tot, c1, hs)
        nc.vector.tensor_add(otv[:, :, 4 * i], sf[i], tot)   # fp32 + bf16 -> fp32

    # ---- off-diagonals: plain bf16 Gram --------------------------------
    offd = {}
    for (i, k) in [(0, 1), (0, 2), (1, 2)]:
        g = w(tag="gg"); h = w(tag="gh"); g2 = w(tag="gg2"); h2 = w(tag="gh2")
        nc.vector.tensor_mul(g, M[i][0], M[k][0])
        nc.vector.tensor_mul(h, M[i][1], M[k][1])
        nc.vector.tensor_add(g2, g, h)
        nc.vector.tensor_mul(h2, M[i][2], M[k][2])
        cik = w(tag=f"cov{i}{k}")
        nc.vector.tensor_add(cik, g2, h2)
        offd[(i, k)] = cik

    # off-diagonals + symmetric duplicates via ScalarE copies (cast to fp32)
    for (i, k), cik in offd.items():
        nc.scalar.copy(out=otv[:, :, 3 * i + k], in_=cik)
        nc.scalar.copy(out=otv[:, :, 3 * k + i], in_=cik)

    nc.sync.dma_start(out=ov[:, 9 * t0:9 * (t0 + f)], in_=ot)


def _pad_and_shard(quaternion, log_scale):
    n = quaternion.shape[0]
    pad = N_CORES * NPC - n
    if pad:
        qpad = np.tile(np.array([1, 0, 0, 0], np.float32), (pad, 1))
        lpad = np.zeros((pad, 3), np.float32)
        quaternion = np.concatenate([quaternion, qpad], axis=0)
        log_scale = np.concatenate([log_scale, lpad], axis=0)
    in_maps = []
    for i in range(N_CORES):
        sl = slice(i * NPC, (i + 1) * NPC)
        in_maps.append({
            "q": np.ascontiguousarray(quaternion[sl]),
            "ls": np.ascontiguousarray(log_scale[sl]),
        })
    return in_maps


def kernel_with_stats(quaternion, log_scale, trace=False):
    quaternion = np.asarray(quaternion, dtype=np.float32)
    log_scale = np.asarray(log_scale, dtype=np.float32)
    n = quaternion.shape[0]
    nc = _build()
    in_maps = _pad_and_shard(quaternion, log_scale)
    res = run_bass_kernel_spmd(nc, in_maps, core_ids=list(range(N_CORES)), trace=trace)
    out = np.concatenate([r["cov"] for r in res.results], axis=0)[:n]
    return out, res


def kernel(quaternion, log_scale):
    out, _ = kernel_with_stats(quaternion, log_scale, trace=False)
    return out
